# revision 59
# baseline (speedup 1.0000x reference)
"""GATv2 (2-layer + linear head) Trainium2 Bass kernel, 8-core SPMD.

Architecture (v2): src-octant edge sharding + dst-major edge layout.

- Core j owns the edges whose src lies in node octant j (12.5k nodes), for
  ALL destinations.  Its gather table (att-prescaled xl rows for its octant)
  has 12544 rows, so int16 gather indices address it directly -- no table
  chunking, no chunk-aligned edge grouping.
- Per core, destinations are sorted by per-octant in-degree and packed into
  128-dst windows; window w holds a [128, K_w] dst-major edge grid (rows =
  dsts, columns = edge slots).  Degree-sorted windows make K_w ~= the max
  in-window degree with ~no padding.  Windows with equal K are batched so
  every DVE op runs on a big uniform [128, NW*K*C] tile.
- xl rows are fetched with one dma_gather per batch (up to ~10k indices per
  call -- the SWDGE ring counts ~num_idxs/16 descriptors, so large calls fit
  the default ring and the 994ns/call descriptor-gen overhead amortizes).
- xr never needs a gather: in dst-major layout it is one row per dst, so the
  xr node-linear runs fused per window (PE matmul from the per-core
  dst-permuted x, activation-engine PSUM->fp16 evacuation) and broadcasts
  over the K edge columns with a 0-stride AP.
- Tables/xr are pre-scaled by att (sign kept, channels sign-sorted), which
  turns  att . leaky_relu(xl+xr)  into  max(v,.2v) over the positive-att
  column range + min(v,.2v) over the negative range, then a log2 fold-tree
  -- every bulk op is an InstTensorScalarPtr (scalar_tensor_tensor), the
  only DVE op family with the 4x fp16 perf mode.
- exp runs on the activation engine with a broadcast (0-stride) input AP,
  directly producing exp(e) replicated over the C channels; masked (padding)
  slots get exp(e-50)~=0 via an additive bias uploaded per slot (which also
  carries a global softmax shift that keeps exp in fp16 range).
- Each core emits per-dst PARTIAL numerators (sum_k exp(e)*xl) and
  denominators (sum_k exp(e)); the host sums partials across the 8 cores,
  normalizes, un-scales by att, applies biases/leaky-relu, and feeds layer 2
  (same edge structure), then the final linear head.  SPMD: one instruction
  stream, all per-core data (permutations, indices, masks) differs only in
  values, never in shape.
"""
import sys
sys.path.insert(0, '/opt/trn_rl_repo')
import numpy as np

P = 128
N = 100000
F = 128
H1 = 64
H2 = 32
NDEV = 8
OCT = N // NDEV            # 12500 src nodes per device octant
NPAD = 100096              # dst rank space (multiple of 128)
NROW = 12544               # gather table rows (= 98 * 128)
NBLK = NROW // P           # 98 table blocks
COLB = 64                  # max edge columns (NW*K) per batch


def _rowmap():
    """table-write column q -> table row (partition-contiguous writes)."""
    q = np.arange(NROW)
    return (q % P) * NBLK + q // P


_ROWMAP = _rowmap()


def _merge_batches(batches, colb):
    """Merge adjacent same-K batches up to colb columns (slot layout is
    unchanged: merged batches stay gcol-contiguous with the same colbase)."""
    out = []
    for (w0, nw, K, gc) in batches:
        if out:
            (pw0, pnw, pK, pgc) = out[-1]
            if pK == K and pw0 + pnw == w0 and (pnw + nw) * K <= colb:
                out[-1] = (pw0, pnw + nw, K, pgc)
                continue
        out.append((w0, nw, K, gc))
    return out


def _split_tail(batches, n=3):
    """Halve the last n batches (slot layout unchanged; shorter pipe drain)."""
    out = list(batches[:-n]) if n else list(batches)
    for (w0, nw, K, gc) in batches[-n:]:
        if nw < 2:
            out.append((w0, nw, K, gc))
            continue
        h = nw // 2
        out.append((w0, h, K, gc))
        out.append((w0 + h, nw - h, K, gc + h * K))
    return out


def _structure(src, dst):
    """Per-core dst-major edge layout with a common cross-core shape.

    Returns (batches, NWIN, GCT, percore) where percore[j] =
    (sigma, eidx[128,8*GCT] int16, base_mask[128,GCT] f32 in {0,-50}).
    """
    percore_raw = []
    csort_all = []
    for j in range(NDEV):
        m = (src // OCT) == j
        s = (src[m] - OCT * j).astype(np.int64)
        d = dst[m].astype(np.int64)
        cnt = np.bincount(d, minlength=NPAD)
        sigma = np.argsort(-cnt, kind="stable")
        csort_all.append(cnt[sigma])
        percore_raw.append((s, d, cnt, sigma))
    csort_all = np.stack(csort_all)          # [8, NPAD]
    K_w = csort_all[:, ::P].max(axis=0)      # [NPAD//P] cross-core window max
    NWIN = int(np.count_nonzero(K_w))
    assert (K_w[:NWIN] > 0).all(), "window K must be sorted desc"

    batches = []                             # (w0, NW, K, gcol)
    gcol = 0
    w = 0
    while w < NWIN:
        K = int(K_w[w])
        w1 = w
        while w1 < NWIN and K_w[w1] == K:
            w1 += 1
        per = max(1, COLB // K)
        a = w
        while a < w1:
            nb = min(per, w1 - a)
            batches.append((a, nb, K, gcol))
            gcol += nb * K
            a += nb
        w = w1
    GCT = gcol
    colbase = np.zeros(NWIN, np.int64)
    for (w0, nw, K, gc) in batches:
        colbase[w0:w0 + nw] = gc + np.arange(nw) * K

    pad_row = int(_ROWMAP[OCT])              # table col OCT is zero-padded
    percore = []
    for j in range(NDEV):
        s, d, cnt, sigma = percore_raw[j]
        rank = np.empty(NPAD, np.int64)
        rank[sigma] = np.arange(NPAD)
        r = rank[d]
        order = np.argsort(r, kind="stable")
        rs = r[order]
        ss = s[order]
        starts = np.r_[0, np.flatnonzero(np.diff(rs)) + 1]
        lens = np.diff(np.r_[starts, len(rs)])
        k = np.arange(len(rs)) - np.repeat(starts, lens)
        w_e = rs // P
        p_e = rs % P
        col = colbase[w_e] + k
        pos = col * P + p_e
        idxflat = np.full(GCT * P, pad_row, np.int16)
        maskflat = np.full(GCT * P, -50.0, np.float32)
        idxflat[pos] = _ROWMAP[ss]
        maskflat[pos] = 0.0
        # pack idx per gather call (= per batch): logical i -> [i%16, i//16]
        eidx = np.zeros((P, 8 * GCT), np.int16)
        arr = idxflat.reshape(GCT, P)
        for (w0, nw, K, gc) in batches:
            cols = nw * K
            a = arr[gc:gc + cols].reshape(cols * 8, 16).T   # [16, cols*8]
            eidx[:, 8 * gc:8 * (gc + cols)] = np.tile(a, (8, 1))
        base_mask = np.ascontiguousarray(maskflat.reshape(GCT, P).T)  # [128, GCT]
        percore.append((sigma, eidx, base_mask))
    return batches, NWIN, GCT, percore


def _pack_idx_mask(eidx, base_mask, batches, shift):
    """Interleave idx + fp16 mask-bias into one int16 upload: per batch,
    [idx cols*8 | mask cols] -> 9 int16 columns per edge column."""
    GCT = base_mask.shape[1]
    out = np.zeros((P, 9 * GCT), np.int16)
    mask16 = (base_mask - shift).astype(np.float16).view(np.int16)
    for (w0, nw, K, gc) in batches:
        cols = nw * K
        o = 9 * gc
        out[:, o:o + 8 * cols] = eidx[:, 8 * gc:8 * (gc + cols)]
        out[:, o + 8 * cols:o + 9 * cols] = mask16[:, gc:gc + cols]
    return out


def _dma_gather_any(gp, out_ap, in_ap, idxs_ap, num_idxs, elem_size,
                    elem_step, queue_num):
    """dma_gather with arbitrary gathered-row byte size (not a multiple of
    256B).  bass.dma_gather asserts elem_size_bytes % 256 == 0, but per the
    Q7 ucode that restriction only applies to transpose mode; non-transpose
    descriptors are byte-granular (only the table row STRIDE must be a
    multiple of 256B).  Emits InstDMAGatherAnt directly."""
    import concourse.mybir as mybir
    import concourse.ap_utils as ap_utils
    assert idxs_ap.dtype == mybir.dt.int16
    assert in_ap.dtype == out_ap.dtype
    assert ap_utils.ap_is_contiguous(in_ap.ap[1:])
    assert ap_utils.ap_is_contiguous(out_ap.ap[1:])
    assert ap_utils.ap_is_contiguous(idxs_ap.ap[1:])
    assert in_ap.ap[-1][1] == out_ap.ap[-1][1] == elem_size
    assert in_ap.ap[0][0] == elem_step
    assert num_idxs % P == 0
    assert out_ap.ap[0][1] * out_ap.ap[1][1] == num_idxs
    stride_bytes = elem_step * mybir.dt.size(in_ap.dtype)
    stride_bytes_256 = stride_bytes // 256
    assert stride_bytes_256 * 256 == stride_bytes and stride_bytes_256 < 256
    _in_ap = gp.lower_ap_dma(in_ap, for_custom_bir_dma=True)
    _idxs_ap = gp.lower_ap(idxs_ap)
    _out_ap = gp.lower_ap(out_ap)
    return gp.add_instruction(
        mybir.InstDMAGatherAnt(
            name=gp.bass.get_next_instruction_name(),
            ins=[*_in_ap, _idxs_ap,
                 gp.lower_val_access(gp.to_reg(num_idxs))],
            outs=[_out_ap],
            transpose=False,
            num_idxs=num_idxs,
            elem_size=elem_size,
            stride_bytes_256=stride_bytes_256,
            gen_mode=0,
            single_packet=True,
            queue_num=queue_num,
            sbuf_tokens_per_rank=0,
            sbuf_free_dim_per_rank=0,
            sbuf_free_dim_pad_per_rank=0,
            sbuf_byte_offset=0,
        ))


def _build_layer(Cin, C, Cp, batches, NWIN, GCT, colb=COLB):
    import concourse.bacc as bacc
    import concourse.mybir as mybir
    import concourse.tile as tile
    from concourse.tile_rust import add_dep_helper

    f32 = mybir.dt.float32
    f16 = mybir.dt.float16
    i16 = mybir.dt.int16
    AL = mybir.AluOpType
    AF = mybir.ActivationFunctionType
    ICT = 8 * GCT

    nc = bacc.Bacc("TRN2", target_bir_lowering=False, debug=False,
                   num_swdge_queues=4, dynamic_dma_scratch_size=16384)
    t_xoT = nc.dram_tensor("xoT", [Cin, NROW], f16, kind="ExternalInput")
    t_xdT = nc.dram_tensor("xdT", [Cin, NWIN * P], f16, kind="ExternalInput")
    t_wlr = nc.dram_tensor("wlr", [Cin, 2 * C], f16, kind="ExternalInput")
    t_bl = nc.dram_tensor("bl", [P, C], f16, kind="ExternalInput")
    t_eidx = nc.dram_tensor("eidx", [P, 9 * GCT], i16, kind="ExternalInput")
    t_out = nc.dram_tensor("out", [P, NWIN * C], f16, kind="ExternalOutput")
    t_den = nc.dram_tensor("den", [P, NWIN], f32, kind="ExternalOutput")
    tab = nc.dram_tensor("tab", [NROW, P], f16, kind="Internal")

    def stt(eng, out, in0, scalar, in1, op0, op1):
        return eng.scalar_tensor_tensor(out=out, in0=in0, scalar=scalar,
                                        in1=in1, op0=op0, op1=op1)

    def tt(out, in0, in1, op):
        return nc.vector.tensor_tensor(out=out, in0=in0, in1=in1, op=op)

    def fold(pool, cur, A, W, B, tag, size, out1):
        """Fold-add axis 2 of cur [P, A, W, B] down to out1 [P, A, 1, B].
        tensor_tensor adds (2x fp16); odd leftovers via tensor_scalar (4x)."""
        if W == 1:
            nc.vector.tensor_scalar(out=out1, in0=cur, scalar1=1.0,
                                    scalar2=None, op0=AL.mult)
            return
        while W > 1:
            h = W // 2
            odd = W - 2 * h
            tw = h + odd
            if tw == 1:
                nxt = out1
            else:
                ft = pool.tile([P, size], f16, tag=tag, name="ft")
                nxt = ft[:, 0:A * tw * B].rearrange(
                    "p (a w b) -> p a w b", a=A, w=tw, b=B)
            tt(nxt[:, :, 0:h, :], cur[:, :, 0:h, :], cur[:, :, h:W - odd, :],
               AL.add)
            if odd:
                nc.vector.tensor_scalar(out=nxt[:, :, h:h + 1, :],
                                        in0=cur[:, :, W - 1:W, :],
                                        scalar1=1.0, scalar2=None, op0=AL.mult)
            cur = nxt
            W = tw

    with tile.TileContext(nc) as tc:
        with tc.tile_pool(name="const", bufs=1) as cp:
            wlr = cp.tile([Cin, 2 * C], f16)
            nc.sync.dma_start(out=wlr[:], in_=t_wlr[:])
            wl = wlr[:, 0:C]
            wr = wlr[:, C:2 * C]
            bl = cp.tile([P, 1, C], f16)
            nc.sync.dma_start(out=bl[:, 0, :], in_=t_bl[:])
            den32 = cp.tile([P, NWIN], f32)

            with tc.tile_pool(name="xl", bufs=3) as xlp, \
                 tc.tile_pool(name="nps", bufs=3, space="PSUM") as npsum, \
                 tc.tile_pool(name="nt", bufs=2) as ntp, \
                 tc.tile_pool(name="ldi", bufs=6) as ip, \
                 tc.tile_pool(name="exg", bufs=(7 if colb == COLB else 6)) as xp, \
                 tc.tile_pool(name="exr2", bufs=3) as rp, \
                 tc.tile_pool(name="ext", bufs=2) as x2p, \
                 tc.tile_pool(name="rps", bufs=2, space="PSUM") as rpsum, \
                 tc.tile_pool(name="ez", bufs=3) as zp, \
                 tc.tile_pool(name="ef", bufs=2) as fp, \
                 tc.tile_pool(name="eex", bufs=(2 if colb == COLB else 3)) as ep2, \
                 tc.tile_pool(name="ewz", bufs=2) as wp, \
                 tc.tile_pool(name="ekf", bufs=2) as kp, \
                 tc.tile_pool(name="eo", bufs=2) as op2:

                # ---------------- xl table pass ----------------
                # chunked tiles; each partition's rows are contiguous in DRAM
                # (row = p*NBLK + b); only the real C columns are written
                HB = 25
                join = nc.sync.nop()

                def table_chunk(ci):
                    b0 = ci * HB
                    hb = min(HB, NBLK - b0)
                    ot = ntp.tile([P, HB, C], f16, tag="ot", name="ot")
                    for blk in range(b0, b0 + hb, 16):
                        kk = min(16, b0 + hb - blk)
                        xt = xlp.tile([Cin, 16 * P], f16, tag="xt", name="xt")
                        nc.scalar.dma_start(out=xt[:, :kk * P],
                                            in_=t_xoT[:, blk * P:(blk + kk) * P])
                        i = 0
                        while i < kk:
                            k4 = min(4, kk - i)
                            ps = npsum.tile([P, 4, C], f32, space="PSUM",
                                            tag="nps", name="ps")
                            for jj in range(k4):
                                nc.tensor.matmul(out=ps[:, jj, :],
                                                 lhsT=xt[:, (i + jj) * P:(i + jj + 1) * P],
                                                 rhs=wl, start=True, stop=True)
                            stt(nc.vector, ot[:, blk - b0 + i:blk - b0 + i + k4, :],
                                ps[:, 0:k4, :], 1.0,
                                bl.broadcast_to((P, k4, C)),
                                AL.mult, AL.add)
                            i += k4
                    wtab = nc.sync.dma_start(
                        out=tab[:, 0:C].rearrange("(p b) c -> p b c", p=P)[:, b0:b0 + hb, :],
                        in_=ot[:, 0:hb, :])
                    add_dep_helper(join.ins, wtab.ins, sync=True,
                                   reason="table rows ready")

                NCHUNKS = (NBLK + HB - 1) // HB

                # ---------------- edge batches ----------------
                # software-pipelined over 4 stages so the in-order DVE/ACT
                # engines always have ready work from an earlier batch
                st = {}

                def s0a_idx(b):  # idx load (hoistable before the table pass)
                    (w0, NW, K, gc) = batches[b]
                    cols = NW * K
                    idx = ip.tile([P, colb * 9], i16, tag="idx", name="idx")
                    nc.sync.dma_start(out=idx[:, :cols * 9],
                                      in_=t_eidx[:, 9 * gc:9 * (gc + cols)])
                    st[b] = dict(idx=idx)

                def s0a(b):  # gather issue (2 steps ahead of use)
                    # HW limit: <=1024 indices per dma_gather call (the SWDGE
                    # descriptor-ring carveout); split into 8-column sub-calls
                    if b not in st:
                        s0a_idx(b)
                    (w0, NW, K, gc) = batches[b]
                    cols = NW * K
                    idx = st[b]["idx"]
                    xg = xp.tile([P, colb, C], f16, tag="xg", name="xg")
                    for jj, j in enumerate(range(0, cols, 8)):
                        cs = min(8, cols - j)
                        g = _dma_gather_any(nc.gpsimd, xg[:, j:j + cs, :],
                                            tab[0:NROW, 0:C],
                                            idx[:, j * 8:(j + cs) * 8],
                                            cs * P, C, P, (b + jj) % 4)
                        add_dep_helper(g.ins, join.ins, sync=True,
                                       reason="gather after table")
                    st[b]["xg"] = xg

                def s0b(b):  # xr pass
                    (w0, NW, K, gc) = batches[b]
                    xr = rp.tile([P, NW, C], f16, tag="xr", name="xr")
                    done = 0
                    while done < NW:
                        nw16 = min(16, NW - done)
                        xt2 = x2p.tile([Cin, 16 * P], f16, tag="xt2", name="xt2")
                        nc.scalar.dma_start(
                            out=xt2[:, :nw16 * P],
                            in_=t_xdT[:, (w0 + done) * P:(w0 + done + nw16) * P])
                        for s8 in range(0, nw16, 8):
                            nw8 = min(8, nw16 - s8)
                            ps2 = rpsum.tile([P, 8, C], f32, space="PSUM",
                                             tag="rps", name="rps")
                            for wi in range(nw8):
                                nc.tensor.matmul(
                                    out=ps2[:, wi, :],
                                    lhsT=xt2[:, (s8 + wi) * P:(s8 + wi + 1) * P],
                                    rhs=wr, start=True, stop=True)
                            nc.scalar.activation(
                                out=xr[:, done + s8:done + s8 + nw8, :],
                                in_=ps2[:, 0:nw8, :], func=AF.Copy)
                        done += nw16
                    st[b]["xr"] = xr

                def s1(b):  # z = xg + xr, leaky-relu ranges on ACT
                    (w0, NW, K, gc) = batches[b]
                    cols = NW * K
                    xg = st[b]["xg"]
                    xg4 = xg[:, 0:cols, :].rearrange("p (w k) c -> p w k c", k=K)
                    z = zp.tile([P, colb, C], f16, tag="z", name="z")
                    z4 = z[:, 0:cols, :].rearrange("p (w k) c -> p w k c", k=K)
                    xrb = st[b]["xr"][:].rearrange("p w (o c) -> p w o c", o=1) \
                                        .broadcast_to((P, NW, K, C))
                    tt(z4, xg4, xrb, AL.add)
                    # +att columns contribute Prelu(v); -att columns -Prelu(-v)
                    zc = z[:, 0:cols, :]
                    if Cp > 0:
                        nc.scalar.activation(out=zc[:, :, 0:Cp],
                                             in_=zc[:, :, 0:Cp],
                                             func=AF.Prelu, alpha=0.2)
                    if Cp < C:
                        nc.scalar.activation(out=zc[:, :, Cp:C],
                                             in_=zc[:, :, Cp:C],
                                             func=AF.Prelu, alpha=0.2,
                                             scale=-1.0)
                    st[b]["z"] = z

                def s2(b):  # fold C -> e, add mask bias, exp-broadcast
                    (w0, NW, K, gc) = batches[b]
                    cols = NW * K
                    z = st[b]["z"]
                    mb = st[b]["idx"][:, cols * 8:cols * 9].bitcast(f16)
                    zc = z[:, 0:cols, :]
                    em = fp.tile([P, colb], f16, tag="em", name="em")
                    spos = fp.tile([P, colb], f16, tag="spos", name="spos")
                    sneg = fp.tile([P, colb], f16, tag="sneg", name="sneg")
                    FCS = colb * (C // 2 + 1)
                    if Cp > 0:
                        fold(fp, zc[:, :, 0:Cp].rearrange("p a (w o) -> p a w o", o=1),
                             cols, Cp, 1, "fc", FCS,
                             spos[:, 0:cols].rearrange("p (a w o) -> p a w o", w=1, o=1))
                    if Cp < C:
                        fold(fp, zc[:, :, Cp:C].rearrange("p a (w o) -> p a w o", o=1),
                             cols, C - Cp, 1, "fc", FCS,
                             sneg[:, 0:cols].rearrange("p (a w o) -> p a w o", w=1, o=1))
                    if Cp == C:
                        tt(em[:, 0:cols], spos[:, 0:cols], mb, AL.add)
                    elif Cp == 0:
                        tt(em[:, 0:cols], mb, sneg[:, 0:cols], AL.subtract)
                    else:
                        tt(spos[:, 0:cols], spos[:, 0:cols], sneg[:, 0:cols],
                           AL.subtract)
                        tt(em[:, 0:cols], spos[:, 0:cols], mb, AL.add)
                    exr = ep2.tile([P, colb, C], f16, tag="exr", name="exr")
                    emb = em[:, 0:cols].rearrange("p (g o) -> p g o", o=1) \
                                       .broadcast_to((P, cols, C))
                    nc.scalar.activation(out=exr[:, 0:cols, :], in_=emb,
                                         func=AF.Exp)
                    st[b]["exr"] = exr

                def s3(b):  # denominator, weighted numerator, write out
                    (w0, NW, K, gc) = batches[b]
                    cols = NW * K
                    xg = st[b]["xg"]
                    exr = st[b]["exr"]
                    xg4 = xg[:, 0:cols, :].rearrange("p (w k) c -> p w k c", k=K)
                    exr4 = exr[:, 0:cols, :].rearrange("p (w k) c -> p w k c", k=K)
                    nc.vector.tensor_reduce(out=den32[:, w0:w0 + NW],
                                            in_=exr4[:, :, :, 0:1],
                                            axis=mybir.AxisListType.XY,
                                            op=AL.add)
                    outt = op2.tile([P, NW, C], f16, tag="outt", name="outt")
                    out4 = outt[:].rearrange("p w (o c) -> p w o c", o=1)
                    wz = wp.tile([P, colb, C], f16, tag="wz", name="wz")
                    tt(wz[:, 0:cols, :], xg[:, 0:cols, :], exr[:, 0:cols, :],
                       AL.mult)
                    wz4 = wz[:, 0:cols, :].rearrange("p (w k) c -> p w k c", k=K)
                    fold(kp, wz4, NW, K, C, "kf", (2 * colb // 3 + 1) * C, out4)
                    nc.sync.dma_start(out=t_out[:, w0 * C:(w0 + NW) * C],
                                      in_=outt[:])
                    del st[b]

                nb = len(batches)
                for b in range(min(4, nb)):
                    s0a_idx(b)
                for ci in range(NCHUNKS):
                    table_chunk(ci)
                for step in range(nb + 5):
                    if step < nb:
                        s0a(step)
                    if 2 <= step < nb + 2:
                        s0b(step - 2)
                    if 3 <= step < nb + 3:
                        s1(step - 3)
                    if 4 <= step < nb + 4:
                        s2(step - 4)
                    if step >= 5:
                        s3(step - 5)
                nc.sync.dma_start(out=t_den[:], in_=den32[:])
    nc.compile()
    return nc


_CACHE = {}


def _prep_weights(W_l, b_l, W_r, b_r, att):
    """att-prescaled, sign-sorted weights; returns device arrays + recovery."""
    att = np.asarray(att, np.float64)
    perm = np.argsort(-att, kind="stable")
    attp = att[perm]
    Cp = int((attp > 0).sum())
    wl = (np.asarray(W_l, np.float64)[:, perm] * attp).astype(np.float16)
    wr = (np.asarray(W_r, np.float64)[:, perm] * attp).astype(np.float16)
    bsum = (np.asarray(b_l, np.float64) + np.asarray(b_r, np.float64))[perm] * attp
    bl = np.tile(bsum.astype(np.float16)[None, :], (P, 1))
    return perm, attp, Cp, wl, wr, bl


def _sample_shift(x_all, src, dst, W_l, b_l, W_r, b_r, att, rng):
    n = len(src)
    take = min(60000, n)
    sel = rng.choice(n, take, replace=False)
    xs = x_all[src[sel]]
    xd = x_all[dst[sel]]
    z = (xs @ W_l + (b_l + b_r)) + (xd @ W_r)
    z = np.where(z > 0, z, 0.2 * z)
    e = z @ att
    return float(max(0.0, e.max() - 6.0))


def _run_layer(nc, x_all, percore, batches, NWIN, GCT,
               W_l, b_l, W_r, b_r, att, shift):
    from concourse import bass_utils
    perm, attp, Cp, wl, wr, bl = _prep_weights(W_l, b_l, W_r, b_r, att)
    Cin = x_all.shape[1]
    C = len(attp)
    xf = x_all.astype(np.float16)
    in_maps = []
    for j in range(NDEV):
        sigma, eidx, base_mask = percore[j]
        xo = np.zeros((Cin, NROW), np.float16)
        xo[:, :OCT] = xf[OCT * j:OCT * (j + 1)].T
        xd = np.ascontiguousarray(xf[sigma[:NWIN * P]].T)
        in_maps.append(dict(
            xoT=xo, xdT=xd, wlr=np.concatenate([wl, wr], axis=1), bl=bl,
            eidx=_pack_idx_mask(eidx, base_mask, batches, shift)))
    res = bass_utils.run_bass_kernel_spmd(nc, in_maps, core_ids=list(range(NDEV)))
    num_acc = np.zeros((NPAD, C), np.float64)
    den_acc = np.zeros(NPAD, np.float64)
    for j in range(NDEV):
        sigma = percore[j][0]
        nodes = sigma[:NWIN * P]
        numj = res.results[j]["out"].reshape(P, NWIN, C).transpose(1, 0, 2) \
                                    .reshape(NWIN * P, C)
        denj = res.results[j]["den"].reshape(P, NWIN).T.reshape(NWIN * P)
        num_acc[nodes] += numj
        den_acc[nodes] += denj
    val = num_acc[:N] / den_acc[:N, None] / attp
    out = np.empty((N, C), np.float64)
    out[:, perm] = val
    return out, res.exec_time_ns


def kernel(x, edge_index, W1l, b1l, W1r, b1r, att1, bias1,
           W2l, b2l, W2r, b2r, att2, bias2, Wlin, blin):
    x = np.asarray(x, np.float32)
    edge_index = np.asarray(edge_index)
    loops = np.arange(N, dtype=np.int64)
    src = np.concatenate([edge_index[0].astype(np.int64), loops])
    dst = np.concatenate([edge_index[1].astype(np.int64), loops])

    batches, NWIN, GCT, percore = _structure(src, dst)
    batchesA = _split_tail(batches, 2)
    batchesB = _split_tail(_merge_batches(batches, 3 * COLB // 2), 3)
    Cp1 = _prep_weights(W1l, b1l, W1r, b1r, att1)[2]
    Cp2 = _prep_weights(W2l, b2l, W2r, b2r, att2)[2]

    key = ("v2", NWIN, GCT, Cp1, Cp2, tuple(b[2] for b in batches))
    if key not in _CACHE:
        _CACHE[key] = (
            _build_layer(F, H1, Cp1, batchesA, NWIN, GCT),
            _build_layer(H1, H2, Cp2, batchesB, NWIN, GCT, colb=3 * COLB // 2),
        )
    ncA, ncB = _CACHE[key]

    rng = np.random.default_rng(12345)
    x64 = x.astype(np.float64)
    s1 = _sample_shift(x64, src, dst, np.asarray(W1l, np.float64),
                       np.asarray(b1l, np.float64), np.asarray(W1r, np.float64),
                       np.asarray(b1r, np.float64), np.asarray(att1, np.float64),
                       rng)
    val1, tA = _run_layer(ncA, x, percore, batchesA, NWIN, GCT,
                          W1l, b1l, W1r, b1r, att1, s1)
    h_pre = val1 - np.asarray(b1r, np.float64) + np.asarray(bias1, np.float64)
    h = np.where(h_pre > 0, h_pre, 0.01 * h_pre)

    s2 = _sample_shift(h, src, dst, np.asarray(W2l, np.float64),
                       np.asarray(b2l, np.float64), np.asarray(W2r, np.float64),
                       np.asarray(b2r, np.float64), np.asarray(att2, np.float64),
                       rng)
    val2, tB = _run_layer(ncB, h.astype(np.float32), percore, batchesB, NWIN, GCT,
                          W2l, b2l, W2r, b2r, att2, s2)
    h2 = val2 - np.asarray(b2r, np.float64) + np.asarray(bias2, np.float64)
    out = h2 @ np.asarray(Wlin, np.float64) + np.asarray(blin, np.float64)

    kernel._last_exec_ns = (tA, tB)
    return out.reshape(-1).astype(np.float32)
